# revision 10
# baseline (speedup 1.0000x reference)
"""Trainium2 Bass kernel for DisentangledSpatialSA.

Reference computation (per batch b, with C=256, IC=128, N=64*64=4096):
    qkv = w_qkv @ x + b_qkv                    # [384, N]
    q, k, v = qkv split into 3 x [IC, N]
    k -= mean_n(k); q -= mean_n(q)             # per-channel spatial centering
    pw[i, j] = sum_c k[c, i] * q[c, j]
    pw = softmax(pw / (sqrt(IC) * TEMP), axis=j)
    y[c, i] = sum_j pw[i, j] * v[c, j]
    out = x + w_out @ y + b_out

Mathematical simplifications used (exact up to softmax shift invariance):
  - q centering and q/k biases cancel inside the row softmax (terms constant
    over j, or removed by k's own centering), so only k is centered and only
    v's bias is applied.
  - softmax max-subtraction is skipped: logits are ~N(0, 0.5), safely inside
    fp32 exp range.
  - y normalization (divide by row sums of exp) is applied after the PV
    matmul: y = (V e) / s.

Sharding: data-parallel over batch, one batch element per NeuronCore (8).

Per-core layout (everything channel-major, spatial flattened to n=4096):
  S_t[j, i] = sum_c k~[c, i] q[c, j] computed as matmul(lhsT=q-tile[c, 128j],
  rhs=k~[c, 512i]) so softmax sums over j land on the partition axis; they are
  computed with a bf16 pairwise tree on VectorE plus one
  gpsimd.partition_all_reduce (which also broadcasts across partitions).
  PV uses lhsT = v^T tiles (built by DMA transpose of bf16 v), rhs = exp tile.
"""

import numpy as np

import concourse.bacc as bacc
import concourse.bass as bass
import concourse.tile as tile
from concourse import mybir
from concourse import bass_isa
from concourse.bass_utils import run_bass_kernel_spmd

F32 = mybir.dt.float32
F32R = mybir.dt.float32r
BF16 = mybir.dt.bfloat16

CH = 256
IC = 128
N = 4096
TEMP = 0.05
SCALE = 1.0 / (np.sqrt(np.float32(IC)) * TEMP)  # applied inside exp

P = 128          # partitions
IMW = 1024       # i-macro tile width (query-free dim per attention pass)
NMACRO = N // IMW
NJ = N // P      # 32 key tiles
MMF = 512        # max moving free dim per matmul


def build_bass() -> bass.Bass:
    nc = bacc.Bacc("TRN2", target_bir_lowering=False, debug=False, num_devices=8)

    x_d = nc.dram_tensor("x", [CH, N], F32, kind="ExternalInput")
    wqkvT_d = nc.dram_tensor("wqkvT", [CH, 3 * IC], F32, kind="ExternalInput")
    bv_d = nc.dram_tensor("bv", [IC, 1], F32, kind="ExternalInput")
    woutT_d = nc.dram_tensor("woutT", [IC, CH], F32, kind="ExternalInput")
    bout_d = nc.dram_tensor("bout", [CH, 1], F32, kind="ExternalInput")
    out_d = nc.dram_tensor("out", [CH, N], F32, kind="ExternalOutput")

    with tile.TileContext(nc) as tc:
        with (
            tc.tile_pool(name="big", bufs=1) as big,          # long-lived SBUF
            tc.tile_pool(name="small", bufs=1) as small,      # weights/bias
            tc.tile_pool(name="ework", bufs=4) as ework,      # exp tiles
            tc.tile_pool(name="tree", bufs=2) as treep,       # softmax-sum tree
            tc.tile_pool(name="norm", bufs=2) as normp,       # sums/recip/y_sb
            tc.tile_pool(name="outp", bufs=4) as outp,        # output staging
            tc.tile_pool(name="spsum", bufs=2, space="PSUM") as spsum,  # 4 banks
            tc.tile_pool(name="ypsum", bufs=2, space="PSUM") as ypsum,  # 4 banks
        ):
            # ---------- load inputs ----------
            X = []
            for cchunk in range(2):
                xt = big.tile([P, N], F32, tag=f"x{cchunk}")
                nc.sync.dma_start(out=xt, in_=x_d[cchunk * P:(cchunk + 1) * P, :])
                X.append(xt)
            W = []
            for cchunk in range(2):
                wt = small.tile([P, 3 * IC], F32, tag=f"w{cchunk}")
                nc.sync.dma_start(out=wt, in_=wqkvT_d[cchunk * P:(cchunk + 1) * P, :])
                W.append(wt)
            woutT = small.tile([IC, CH], F32, tag="woutT")
            nc.sync.dma_start(out=woutT, in_=woutT_d[:, :])
            bv = small.tile([IC, 1], F32, tag="bv")
            nc.sync.dma_start(out=bv, in_=bv_d[:, :])
            bout = []
            for oc in range(2):
                bt = small.tile([P, 1], F32, tag=f"bout{oc}")
                nc.sync.dma_start(out=bt, in_=bout_d[oc * P:(oc + 1) * P, :])
                bout.append(bt)

            # fp32r matmul operands must be produced by compute ops that round
            # to fp32r, so round the DMA-loaded X/W once on VectorE.
            Xr = []
            Wr = []
            for cchunk in range(2):
                xr = big.tile([P, N], F32R, tag=f"xr{cchunk}")
                nc.vector.tensor_copy(xr, X[cchunk])
                Xr.append(xr)
                wr = small.tile([P, 3 * IC], F32R, tag=f"wr{cchunk}")
                nc.vector.tensor_copy(wr, W[cchunk])
                Wr.append(wr)
            woutTr = small.tile([IC, CH], F32R, tag="woutTr")
            nc.vector.tensor_copy(woutTr, woutT)

            # ---------- QKV projection (f32r, full speed at free=512) ----------
            q_sb = big.tile([P, N], F32R, tag="q")
            k_sb = big.tile([P, N], F32, tag="k")
            v_bf = big.tile([P, N], BF16, tag="v")
            for m in range(3):
                for nt in range(N // MMF):
                    ps = ypsum.tile([P, MMF], F32, tag="ypsum")
                    sl = slice(nt * MMF, (nt + 1) * MMF)
                    for cchunk in range(2):
                        nc.tensor.matmul(
                            ps,
                            Wr[cchunk][:, m * IC:(m + 1) * IC],
                            Xr[cchunk][:, sl],
                            start=(cchunk == 0),
                            stop=(cchunk == 1),
                        )
                    if m == 0:
                        nc.vector.tensor_copy(q_sb[:, sl], ps)
                    elif m == 1:
                        nc.vector.tensor_copy(k_sb[:, sl], ps)
                    else:
                        nc.scalar.activation(
                            out=v_bf[:, sl], in_=ps,
                            func=mybir.ActivationFunctionType.Identity,
                            bias=bv, scale=1.0,
                        )

            # ---------- center k over spatial axis ----------
            ksum2 = small.tile([P, 2], F32, tag="ksum2")
            nc.vector.tensor_reduce(
                out=ksum2,
                in_=k_sb.rearrange("p (a b) -> p a b", a=2),
                axis=mybir.AxisListType.X,
                op=mybir.AluOpType.add,
            )
            kneg = small.tile([P, 1], F32, tag="kneg")
            nc.vector.tensor_reduce(
                out=kneg, in_=ksum2, axis=mybir.AxisListType.X, op=mybir.AluOpType.add
            )
            nc.vector.tensor_scalar_mul(kneg, kneg, -1.0 / N)
            kc_sb = big.tile([P, N], F32R, tag="kc")
            for h in range(2):
                sl = slice(h * (N // 2), (h + 1) * (N // 2))
                nc.scalar.activation(
                    out=kc_sb[:, sl], in_=k_sb[:, sl],
                    func=mybir.ActivationFunctionType.Identity,
                    bias=kneg, scale=1.0,
                )

            # ---------- v^T tiles via DMA transpose (bf16) ----------
            vt = big.tile([P, NJ, IC], BF16, tag="vt")
            for jt in range(NJ):
                nc.sync.dma_start_transpose(
                    out=vt[:, jt, :], in_=v_bf[:, jt * P:(jt + 1) * P]
                )

            # ---------- attention ----------
            for im in range(NMACRO):
                isl = slice(im * IMW, (im + 1) * IMW)
                yps = ypsum.tile([P, IMW], F32, tag="ypsum")
                levels: list = [None] * 8
                for jt in range(NJ):
                    sps = spsum.tile([P, IMW], F32, tag="s")
                    for h in range(IMW // MMF):
                        nc.tensor.matmul(
                            sps[:, h * MMF:(h + 1) * MMF],
                            q_sb[:, jt * P:(jt + 1) * P],
                            kc_sb[:, im * IMW + h * MMF: im * IMW + (h + 1) * MMF],
                            start=True,
                            stop=True,
                        )
                    e = ework.tile([P, IMW], BF16, tag="e")
                    nc.scalar.activation(
                        out=e, in_=sps,
                        func=mybir.ActivationFunctionType.Exp,
                        scale=float(SCALE),
                    )
                    for h in range(IMW // MMF):
                        nc.tensor.matmul(
                            yps[:, h * MMF:(h + 1) * MMF],
                            vt[:, jt, :],
                            e[:, h * MMF:(h + 1) * MMF],
                            start=(jt == 0),
                            stop=(jt == NJ - 1),
                        )
                    # pairwise bf16 tree accumulating the softmax denominators
                    cur, lvl = e, 0
                    with nc.allow_low_precision("softmax denom tree in bf16"):
                        while levels[lvl] is not None:
                            nxt = treep.tile([P, IMW], BF16, tag=f"tree{lvl}")
                            nc.vector.tensor_add(nxt, levels[lvl], cur)
                            levels[lvl] = None
                            cur = nxt
                            lvl += 1
                    levels[lvl] = cur
                total = levels[5]
                assert total is not None and all(
                    levels[i] is None for i in range(8) if i != 5
                )
                # sum over the in-tile j (partition axis), broadcast to all rows
                s_bc = normp.tile([P, IMW], F32, tag="sbc")
                nc.gpsimd.partition_all_reduce(
                    s_bc, total, channels=P, reduce_op=bass_isa.ReduceOp.add
                )
                r_bc = normp.tile([P, IMW], F32, tag="rbc")
                nc.vector.reciprocal(r_bc, s_bc)
                y_sb = normp.tile([P, IMW], F32R, tag="ysb")
                nc.vector.tensor_mul(y_sb, yps, r_bc)

                # ---------- output projection + residual ----------
                for oc in range(2):
                    pps = ypsum.tile([P, IMW], F32, tag="ypsum")
                    for h in range(IMW // MMF):
                        nc.tensor.matmul(
                            pps[:, h * MMF:(h + 1) * MMF],
                            woutTr[:, oc * P:(oc + 1) * P],
                            y_sb[:, h * MMF:(h + 1) * MMF],
                            start=True,
                            stop=True,
                        )
                    osb = outp.tile([P, IMW], F32, tag="osb")
                    nc.scalar.activation(
                        out=osb, in_=pps,
                        func=mybir.ActivationFunctionType.Identity,
                        bias=bout[oc], scale=1.0,
                    )
                    nc.vector.tensor_add(osb, osb, X[oc][:, isl])
                    nc.sync.dma_start(
                        out=out_d[oc * P:(oc + 1) * P, isl], in_=osb
                    )
    nc.compile()
    return nc


_CACHED_NC = None


def _get_nc():
    global _CACHED_NC
    if _CACHED_NC is None:
        _CACHED_NC = build_bass()
    return _CACHED_NC


def _prep_in_maps(x, w_qkv, b_qkv, w_out, b_out):
    xs = np.ascontiguousarray(np.asarray(x, np.float32).reshape(8, CH, N))
    wqkvT = np.ascontiguousarray(np.asarray(w_qkv, np.float32).T)
    bv = np.ascontiguousarray(
        np.asarray(b_qkv, np.float32)[2 * IC:3 * IC].reshape(IC, 1)
    )
    woutT = np.ascontiguousarray(np.asarray(w_out, np.float32).T)
    bout = np.ascontiguousarray(np.asarray(b_out, np.float32).reshape(CH, 1))
    return [
        {
            "x": np.ascontiguousarray(xs[i]),
            "wqkvT": wqkvT,
            "bv": bv,
            "woutT": woutT,
            "bout": bout,
        }
        for i in range(8)
    ]


def kernel(x, w_qkv, b_qkv, w_out, b_out, _trace=False, _trace_kwargs=None):
    nc = _get_nc()
    in_maps = _prep_in_maps(x, w_qkv, b_qkv, w_out, b_out)
    res = run_bass_kernel_spmd(
        nc, in_maps, core_ids=list(range(8)), trace=_trace,
        **(_trace_kwargs or {}),
    )
    out = np.stack([res.results[i]["out"] for i in range(8)])
    out = out.reshape(8, CH, 64, 64).astype(np.float32)
    if _trace:
        return out, res
    return out


if __name__ == "__main__":
    rng = np.random.default_rng(0)
    x = rng.standard_normal((8, CH, 64, 64), dtype=np.float32)
    w_qkv = (rng.standard_normal((3 * IC, CH), dtype=np.float32) * 0.01)
    b_qkv = (rng.standard_normal((3 * IC,), dtype=np.float32) * 0.01)
    w_out = (rng.standard_normal((CH, IC), dtype=np.float32) * 0.01)
    b_out = (rng.standard_normal((CH,), dtype=np.float32) * 0.01)
    o = kernel(x, w_qkv=w_qkv, b_qkv=b_qkv, w_out=w_out, b_out=b_out)
    print(o.shape, o.dtype)


# revision 11
# speedup vs baseline: 1.0754x; 1.0754x over previous
"""Trainium2 Bass kernel for DisentangledSpatialSA.

Reference computation (per batch b, with C=256, IC=128, N=64*64=4096):
    qkv = w_qkv @ x + b_qkv                    # [384, N]
    q, k, v = qkv split into 3 x [IC, N]
    k -= mean_n(k); q -= mean_n(q)             # per-channel spatial centering
    pw[i, j] = sum_c k[c, i] * q[c, j]
    pw = softmax(pw / (sqrt(IC) * TEMP), axis=j)
    y[c, i] = sum_j pw[i, j] * v[c, j]
    out = x + w_out @ y + b_out

Simplifications used (exact up to softmax shift invariance):
  - q centering and the q/k biases cancel inside the row softmax, so only k
    is centered and only v's bias is applied.
  - softmax max-subtraction is skipped: logits are ~N(0, 0.5), safely inside
    fp32 exp range.
  - normalization is applied after the PV matmul: y = (V e) / s, with the
    row sums s computed by a bf16 pairwise tree on VectorE plus one
    gpsimd.partition_all_reduce (which also broadcasts across partitions).

Sharding: data-parallel over batch, one batch element per NeuronCore (8).

Layout: everything channel-major with spatial flattened (n = 4096).
S_t[j, i] tiles are built with keys j on partitions (lhsT = q-tile, rhs = k~),
so the softmax denominators are partition-axis sums; PV uses lhsT = v^T tiles
(DMA transpose of bf16 v) and rhs = exp(S_t).
"""

import numpy as np

import concourse.bacc as bacc
import concourse.bass as bass
import concourse.tile as tile
from concourse import mybir
from concourse import bass_isa
from concourse.bass_utils import run_bass_kernel_spmd

F32 = mybir.dt.float32
F32R = mybir.dt.float32r
BF16 = mybir.dt.bfloat16

CH = 256
IC = 128
N = 4096
TEMP = 0.05
SCALE = 1.0 / (np.sqrt(np.float32(IC)) * TEMP)  # applied inside exp

P = 128          # partitions
IMW = 1024       # i-macro tile width (query free dim per attention pass)
NMACRO = N // IMW
NJ = N // P      # 32 key tiles
MMF = 512        # max moving free dim per matmul


def build_bass() -> bass.Bass:
    nc = bacc.Bacc("TRN2", target_bir_lowering=False, debug=False, num_devices=8)

    # fp32r-typed external inputs: bits are fp32; fp32r lets matmuls consume
    # them at full (1 cycle/row) rate without an on-chip rounding pass.
    x_d = nc.dram_tensor("x", [CH, N], F32R, kind="ExternalInput")
    wqkvT_d = nc.dram_tensor("wqkvT", [CH, 3 * IC], F32R, kind="ExternalInput")
    bv_d = nc.dram_tensor("bv", [IC, 1], F32, kind="ExternalInput")
    woutT_d = nc.dram_tensor("woutT", [IC, CH], F32R, kind="ExternalInput")
    bout_d = nc.dram_tensor("bout", [CH, 1], F32, kind="ExternalInput")
    out_d = nc.dram_tensor("out", [CH, N], F32, kind="ExternalOutput")

    with tile.TileContext(nc) as tc:
        with (
            tc.tile_pool(name="big", bufs=1) as big,          # long-lived SBUF
            tc.tile_pool(name="small", bufs=1) as small,      # weights/bias
            tc.tile_pool(name="ework", bufs=4) as ework,      # exp tiles
            tc.tile_pool(name="tree", bufs=2) as treep,       # softmax-sum tree
            tc.tile_pool(name="norm", bufs=2) as normp,       # sums/recip
            tc.tile_pool(name="outp", bufs=4) as outp,        # output staging
            tc.tile_pool(name="spsum", bufs=2, space="PSUM") as spsum,  # 4 banks
            tc.tile_pool(name="ypsum", bufs=2, space="PSUM") as ypsum,  # 4 banks
        ):
            # ---------- load inputs (x chunked so compute starts early) ----
            X = []
            for cchunk in range(2):
                xt = big.tile([P, N], F32R, tag=f"x{cchunk}")
                for h in range(4):
                    sl = slice(h * (N // 4), (h + 1) * (N // 4))
                    nc.sync.dma_start(
                        out=xt[:, sl], in_=x_d[cchunk * P:(cchunk + 1) * P, sl]
                    )
                X.append(xt)
            W = []
            for cchunk in range(2):
                wt = small.tile([P, 3 * IC], F32R, tag=f"w{cchunk}")
                nc.sync.dma_start(out=wt, in_=wqkvT_d[cchunk * P:(cchunk + 1) * P, :])
                W.append(wt)
            woutT = small.tile([IC, CH], F32R, tag="woutT")
            nc.sync.dma_start(out=woutT, in_=woutT_d[:, :])
            bv = small.tile([IC, 1], F32, tag="bv")
            nc.sync.dma_start(out=bv, in_=bv_d[:, :])
            bout = []
            for oc in range(2):
                bt = small.tile([P, 1], F32, tag=f"bout{oc}")
                nc.sync.dma_start(out=bt, in_=bout_d[oc * P:(oc + 1) * P, :])
                bout.append(bt)

            # ---------- QKV projection ----------
            q_sb = big.tile([P, N], BF16, tag="q")
            k_sb = big.tile([P, N], F32, tag="k")
            v_bf = big.tile([P, N], BF16, tag="v")
            for m in range(3):
                for nt in range(N // MMF):
                    ps = ypsum.tile([P, MMF], F32, tag="ypsum")
                    sl = slice(nt * MMF, (nt + 1) * MMF)
                    for cchunk in range(2):
                        nc.tensor.matmul(
                            ps,
                            W[cchunk][:, m * IC:(m + 1) * IC],
                            X[cchunk][:, sl],
                            start=(cchunk == 0),
                            stop=(cchunk == 1),
                        )
                    if m == 0:
                        with nc.allow_low_precision("q used in bf16 logits"):
                            nc.vector.tensor_copy(q_sb[:, sl], ps)
                    elif m == 1:
                        nc.vector.tensor_copy(k_sb[:, sl], ps)
                    else:
                        nc.scalar.activation(
                            out=v_bf[:, sl], in_=ps,
                            func=mybir.ActivationFunctionType.Identity,
                            bias=bv, scale=1.0,
                        )

            # ---------- center k over spatial axis (write bf16) ----------
            ksum2 = small.tile([P, 2], F32, tag="ksum2")
            nc.vector.tensor_reduce(
                out=ksum2,
                in_=k_sb.rearrange("p (a b) -> p a b", a=2),
                axis=mybir.AxisListType.X,
                op=mybir.AluOpType.add,
            )
            kneg = small.tile([P, 1], F32, tag="kneg")
            nc.vector.tensor_reduce(
                out=kneg, in_=ksum2, axis=mybir.AxisListType.X, op=mybir.AluOpType.add
            )
            nc.vector.tensor_scalar_mul(kneg, kneg, -1.0 / N)
            kc_sb = big.tile([P, N], BF16, tag="kc")
            for h in range(4):
                sl = slice(h * (N // 4), (h + 1) * (N // 4))
                nc.scalar.activation(
                    out=kc_sb[:, sl], in_=k_sb[:, sl],
                    func=mybir.ActivationFunctionType.Identity,
                    bias=kneg, scale=1.0,
                )

            # ---------- v^T tiles via DMA transpose (bf16) ----------
            vt = big.tile([P, NJ, IC], BF16, tag="vt")
            for jt in range(NJ):
                nc.sync.dma_start_transpose(
                    out=vt[:, jt, :], in_=v_bf[:, jt * P:(jt + 1) * P]
                )

            # ---------- attention (normalized y saved; projection deferred) --
            y_tiles = []
            for im in range(NMACRO):
                yps = ypsum.tile([P, IMW], F32, tag="ypsum")
                levels: list = [None] * 8
                for jt in range(NJ):
                    sps = spsum.tile([P, IMW], F32, tag="s")
                    for h in range(IMW // MMF):
                        nc.tensor.matmul(
                            sps[:, h * MMF:(h + 1) * MMF],
                            q_sb[:, jt * P:(jt + 1) * P],
                            kc_sb[:, im * IMW + h * MMF: im * IMW + (h + 1) * MMF],
                            start=True,
                            stop=True,
                        )
                    e = ework.tile([P, IMW], BF16, tag="e")
                    nc.scalar.activation(
                        out=e, in_=sps,
                        func=mybir.ActivationFunctionType.Exp,
                        scale=float(SCALE),
                    )
                    for h in range(IMW // MMF):
                        nc.tensor.matmul(
                            yps[:, h * MMF:(h + 1) * MMF],
                            vt[:, jt, :],
                            e[:, h * MMF:(h + 1) * MMF],
                            start=(jt == 0),
                            stop=(jt == NJ - 1),
                        )
                    # pairwise bf16 tree for the softmax denominators
                    cur, lvl = e, 0
                    with nc.allow_low_precision("softmax denom tree in bf16"):
                        while levels[lvl] is not None:
                            nxt = treep.tile([P, IMW], BF16, tag=f"tree{lvl}")
                            nc.vector.tensor_add(nxt, levels[lvl], cur)
                            levels[lvl] = None
                            cur = nxt
                            lvl += 1
                    levels[lvl] = cur
                total = levels[5]
                assert total is not None and all(
                    levels[i] is None for i in range(8) if i != 5
                )
                # sum over in-tile j (partition axis), broadcast to all rows
                s_bc = normp.tile([P, IMW], F32, tag="sbc")
                nc.gpsimd.partition_all_reduce(
                    s_bc, total, channels=P, reduce_op=bass_isa.ReduceOp.add
                )
                r_bc = normp.tile([P, IMW], F32, tag="rbc")
                nc.vector.reciprocal(r_bc, s_bc)
                y_sb = big.tile([P, IMW], F32R, tag=f"ysb{im}")
                with nc.allow_low_precision("y normalized into f32r"):
                    nc.vector.tensor_mul(y_sb, yps, r_bc)
                y_tiles.append(y_sb)

            # ---------- output projection + residual + store ----------
            for im in range(NMACRO):
                isl = slice(im * IMW, (im + 1) * IMW)
                for oc in range(2):
                    pps = ypsum.tile([P, IMW], F32, tag="ypsum")
                    for h in range(IMW // MMF):
                        nc.tensor.matmul(
                            pps[:, h * MMF:(h + 1) * MMF],
                            woutT[:, oc * P:(oc + 1) * P],
                            y_tiles[im][:, h * MMF:(h + 1) * MMF],
                            start=True,
                            stop=True,
                        )
                    osb = outp.tile([P, IMW], F32, tag="osb")
                    nc.scalar.activation(
                        out=osb, in_=pps,
                        func=mybir.ActivationFunctionType.Identity,
                        bias=bout[oc], scale=1.0,
                    )
                    nc.vector.tensor_add(osb, osb, X[oc][:, isl].bitcast(F32))
                    nc.sync.dma_start(out=out_d[oc * P:(oc + 1) * P, isl], in_=osb)
    nc.compile()
    return nc


_CACHED_NC = None


def _get_nc():
    global _CACHED_NC
    if _CACHED_NC is None:
        _CACHED_NC = build_bass()
    return _CACHED_NC


def _prep_in_maps(x, w_qkv, b_qkv, w_out, b_out):
    xs = np.ascontiguousarray(np.asarray(x, np.float32).reshape(8, CH, N))
    wqkvT = np.ascontiguousarray(np.asarray(w_qkv, np.float32).T)
    bv = np.ascontiguousarray(
        np.asarray(b_qkv, np.float32)[2 * IC:3 * IC].reshape(IC, 1)
    )
    woutT = np.ascontiguousarray(np.asarray(w_out, np.float32).T)
    bout = np.ascontiguousarray(np.asarray(b_out, np.float32).reshape(CH, 1))
    return [
        {
            "x": np.ascontiguousarray(xs[i]),
            "wqkvT": wqkvT,
            "bv": bv,
            "woutT": woutT,
            "bout": bout,
        }
        for i in range(8)
    ]


def kernel(x, w_qkv, b_qkv, w_out, b_out, _trace=False, _trace_kwargs=None):
    nc = _get_nc()
    in_maps = _prep_in_maps(x, w_qkv, b_qkv, w_out, b_out)
    res = run_bass_kernel_spmd(
        nc, in_maps, core_ids=list(range(8)), trace=_trace,
        **(_trace_kwargs or {}),
    )
    out = np.stack([res.results[i]["out"] for i in range(8)])
    out = out.reshape(8, CH, 64, 64).astype(np.float32)
    if _trace:
        return out, res
    return out


if __name__ == "__main__":
    rng = np.random.default_rng(0)
    x = rng.standard_normal((8, CH, 64, 64), dtype=np.float32)
    w_qkv = (rng.standard_normal((3 * IC, CH), dtype=np.float32) * 0.01)
    b_qkv = (rng.standard_normal((3 * IC,), dtype=np.float32) * 0.01)
    w_out = (rng.standard_normal((CH, IC), dtype=np.float32) * 0.01)
    b_out = (rng.standard_normal((CH,), dtype=np.float32) * 0.01)
    o = kernel(x, w_qkv=w_qkv, b_qkv=b_qkv, w_out=w_out, b_out=b_out)
    print(o.shape, o.dtype)


# revision 13
# speedup vs baseline: 1.2029x; 1.1186x over previous
"""Trainium2 Bass kernel for DisentangledSpatialSA.

Reference computation (per batch b, with C=256, IC=128, N=64*64=4096):
    qkv = w_qkv @ x + b_qkv                    # [384, N]
    q, k, v = qkv split into 3 x [IC, N]
    k -= mean_n(k); q -= mean_n(q)             # per-channel spatial centering
    pw[i, j] = sum_c k[c, i] * q[c, j]
    pw = softmax(pw / (sqrt(IC) * TEMP), axis=j)
    y[c, i] = sum_j pw[i, j] * v[c, j]
    out = x + w_out @ y + b_out

Simplifications used (exact up to softmax shift invariance):
  - q centering and the q/k biases cancel inside the row softmax, so only k
    is centered and only v's bias is applied.
  - softmax max-subtraction is skipped: logits are ~N(0, 0.5), safely inside
    fp32 exp range.
  - normalization is applied after the PV matmul: y = (V e) / s, with the
    row sums s computed by a bf16 pairwise tree on VectorE plus one
    gpsimd.partition_all_reduce (which also broadcasts across partitions).

Sharding: data-parallel over batch, one batch element per NeuronCore (8).

Layout: everything channel-major with spatial flattened (n = 4096).
S_t[j, i] tiles are built with keys j on partitions (lhsT = q-tile, rhs = k~),
so the softmax denominators are partition-axis sums; PV uses lhsT = v^T tiles
(DMA transpose of bf16 v) and rhs = exp(S_t).
"""

import numpy as np

import concourse.bacc as bacc
import concourse.bass as bass
import concourse.tile as tile
from concourse import mybir
from concourse import bass_isa
from concourse.bass_utils import run_bass_kernel_spmd

F32 = mybir.dt.float32
F32R = mybir.dt.float32r
BF16 = mybir.dt.bfloat16

CH = 256
IC = 128
N = 4096
TEMP = 0.05
SCALE = 1.0 / (np.sqrt(np.float32(IC)) * TEMP)  # applied inside exp

P = 128          # partitions
IMW = 1024       # i-macro tile width (query free dim per attention pass)
NMACRO = N // IMW
NJ = N // P      # 32 key tiles
MMF = 512        # max moving free dim per matmul


def build_bass() -> bass.Bass:
    nc = bacc.Bacc("TRN2", target_bir_lowering=False, debug=False, num_devices=8)

    # fp32r-typed external inputs: bits are fp32; fp32r lets matmuls consume
    # them at full (1 cycle/row) rate without an on-chip rounding pass.
    x_d = nc.dram_tensor("x", [CH, N], F32R, kind="ExternalInput")
    wqkvT_d = nc.dram_tensor("wqkvT", [CH, 3 * IC], F32R, kind="ExternalInput")
    bv_d = nc.dram_tensor("bv", [IC, 1], F32, kind="ExternalInput")
    woutT_d = nc.dram_tensor("woutT", [IC, CH], F32R, kind="ExternalInput")
    bout_d = nc.dram_tensor("bout", [CH, 1], F32, kind="ExternalInput")
    out_d = nc.dram_tensor("out", [CH, N], F32, kind="ExternalOutput")

    with tile.TileContext(nc) as tc:
        with (
            tc.tile_pool(name="big", bufs=1) as big,          # long-lived SBUF
            tc.tile_pool(name="small", bufs=1) as small,      # weights/bias
            tc.tile_pool(name="ework", bufs=8) as ework,      # exp tiles
            tc.tile_pool(name="tree", bufs=3) as treep,       # softmax-sum tree
            tc.tile_pool(name="norm", bufs=2) as normp,       # sums/recip
            tc.tile_pool(name="outp", bufs=4) as outp,        # output staging
            tc.tile_pool(name="spsum", bufs=2, space="PSUM") as spsum,  # 4 banks
            tc.tile_pool(name="ypsum", bufs=2, space="PSUM") as ypsum,  # 4 banks
        ):
            # ---------- load inputs (x chunked so compute starts early) ----
            X = []
            for cchunk in range(2):
                xt = big.tile([P, N], F32R, tag=f"x{cchunk}")
                for h in range(4):
                    sl = slice(h * (N // 4), (h + 1) * (N // 4))
                    nc.sync.dma_start(
                        out=xt[:, sl], in_=x_d[cchunk * P:(cchunk + 1) * P, sl]
                    )
                X.append(xt)
            W = []
            for cchunk in range(2):
                wt = small.tile([P, 3 * IC], F32R, tag=f"w{cchunk}")
                nc.sync.dma_start(out=wt, in_=wqkvT_d[cchunk * P:(cchunk + 1) * P, :])
                W.append(wt)
            woutT = small.tile([IC, CH], F32R, tag="woutT")
            nc.sync.dma_start(out=woutT, in_=woutT_d[:, :])
            bv = small.tile([IC, 1], F32, tag="bv")
            nc.sync.dma_start(out=bv, in_=bv_d[:, :])
            bout = []
            for oc in range(2):
                bt = small.tile([P, 1], F32, tag=f"bout{oc}")
                nc.sync.dma_start(out=bt, in_=bout_d[oc * P:(oc + 1) * P, :])
                bout.append(bt)

            # ---------- QKV projection ----------
            q_sb = big.tile([P, N], BF16, tag="q")
            k_sb = big.tile([P, N], F32, tag="k")
            v_bf = big.tile([P, N], BF16, tag="v")
            for m in range(3):
                for nt in range(N // MMF):
                    ps = ypsum.tile([P, MMF], F32, tag="ypsum")
                    sl = slice(nt * MMF, (nt + 1) * MMF)
                    for cchunk in range(2):
                        nc.tensor.matmul(
                            ps,
                            W[cchunk][:, m * IC:(m + 1) * IC],
                            X[cchunk][:, sl],
                            start=(cchunk == 0),
                            stop=(cchunk == 1),
                        )
                    if m == 0:
                        with nc.allow_low_precision("q used in bf16 logits"):
                            nc.vector.tensor_copy(q_sb[:, sl], ps)
                    elif m == 1:
                        nc.vector.tensor_copy(k_sb[:, sl], ps)
                    else:
                        nc.scalar.activation(
                            out=v_bf[:, sl], in_=ps,
                            func=mybir.ActivationFunctionType.Identity,
                            bias=bv, scale=1.0,
                        )

            # ---------- center k over spatial axis (write bf16) ----------
            ksum2 = small.tile([P, 2], F32, tag="ksum2")
            nc.vector.tensor_reduce(
                out=ksum2,
                in_=k_sb.rearrange("p (a b) -> p a b", a=2),
                axis=mybir.AxisListType.X,
                op=mybir.AluOpType.add,
            )
            kneg = small.tile([P, 1], F32, tag="kneg")
            nc.vector.tensor_reduce(
                out=kneg, in_=ksum2, axis=mybir.AxisListType.X, op=mybir.AluOpType.add
            )
            nc.vector.tensor_scalar_mul(kneg, kneg, -1.0 / N)
            kc_sb = big.tile([P, N], BF16, tag="kc")
            for h in range(4):
                sl = slice(h * (N // 4), (h + 1) * (N // 4))
                nc.scalar.activation(
                    out=kc_sb[:, sl], in_=k_sb[:, sl],
                    func=mybir.ActivationFunctionType.Identity,
                    bias=kneg, scale=1.0,
                )

            # ---------- v^T tiles via DMA transpose (bf16) ----------
            vt = big.tile([P, NJ, IC], BF16, tag="vt")
            for jt in range(NJ):
                nc.sync.dma_start_transpose(
                    out=vt[:, jt, :], in_=v_bf[:, jt * P:(jt + 1) * P]
                )

            # ---------- attention (normalized y saved; projection deferred) --
            y_tiles = []
            for im in range(NMACRO):
                yps = ypsum.tile([P, IMW], F32, tag="ypsum")
                levels: list = [None] * 8
                for jt in range(NJ):
                    sps = spsum.tile([P, IMW], F32, tag="s")
                    for h in range(IMW // MMF):
                        nc.tensor.matmul(
                            sps[:, h * MMF:(h + 1) * MMF],
                            q_sb[:, jt * P:(jt + 1) * P],
                            kc_sb[:, im * IMW + h * MMF: im * IMW + (h + 1) * MMF],
                            start=True,
                            stop=True,
                        )
                    e = ework.tile([P, IMW], BF16, tag="e")
                    nc.scalar.activation(
                        out=e, in_=sps,
                        func=mybir.ActivationFunctionType.Exp,
                        scale=float(SCALE),
                    )
                    for h in range(IMW // MMF):
                        nc.tensor.matmul(
                            yps[:, h * MMF:(h + 1) * MMF],
                            vt[:, jt, :],
                            e[:, h * MMF:(h + 1) * MMF],
                            start=(jt == 0),
                            stop=(jt == NJ - 1),
                        )
                    # pairwise bf16 tree for the softmax denominators
                    cur, lvl = e, 0
                    with nc.allow_low_precision("softmax denom tree in bf16"):
                        while levels[lvl] is not None:
                            nxt = treep.tile([P, IMW], BF16, tag=f"tree{lvl}")
                            nc.vector.tensor_add(nxt, levels[lvl], cur)
                            levels[lvl] = None
                            cur = nxt
                            lvl += 1
                    levels[lvl] = cur
                total = levels[5]
                assert total is not None and all(
                    levels[i] is None for i in range(8) if i != 5
                )
                # sum over in-tile j (partition axis), broadcast to all rows
                s_bc = normp.tile([P, IMW], F32, tag="sbc")
                nc.gpsimd.partition_all_reduce(
                    s_bc, total, channels=P, reduce_op=bass_isa.ReduceOp.add
                )
                r_bc = normp.tile([P, IMW], F32, tag="rbc")
                r_scr = normp.tile([P, IMW], F32, tag="rscr")
                nc.vector.reciprocal_approx_accurate(r_bc, s_bc, scratch=r_scr)
                y_sb = big.tile([P, IMW], F32R, tag=f"ysb{im}")
                with nc.allow_low_precision("y normalized into f32r"):
                    nc.vector.tensor_mul(y_sb, yps, r_bc)
                y_tiles.append(y_sb)

            # ---------- output projection + residual + store ----------
            for im in range(NMACRO):
                isl = slice(im * IMW, (im + 1) * IMW)
                for oc in range(2):
                    pps = ypsum.tile([P, IMW], F32, tag="ypsum")
                    for h in range(IMW // MMF):
                        nc.tensor.matmul(
                            pps[:, h * MMF:(h + 1) * MMF],
                            woutT[:, oc * P:(oc + 1) * P],
                            y_tiles[im][:, h * MMF:(h + 1) * MMF],
                            start=True,
                            stop=True,
                        )
                    osb = outp.tile([P, IMW], F32, tag="osb")
                    nc.scalar.activation(
                        out=osb, in_=pps,
                        func=mybir.ActivationFunctionType.Identity,
                        bias=bout[oc], scale=1.0,
                    )
                    nc.vector.tensor_add(osb, osb, X[oc][:, isl].bitcast(F32))
                    nc.sync.dma_start(out=out_d[oc * P:(oc + 1) * P, isl], in_=osb)
    nc.compile()
    return nc


_CACHED_NC = None


def _get_nc():
    global _CACHED_NC
    if _CACHED_NC is None:
        _CACHED_NC = build_bass()
    return _CACHED_NC


def _prep_in_maps(x, w_qkv, b_qkv, w_out, b_out):
    xs = np.ascontiguousarray(np.asarray(x, np.float32).reshape(8, CH, N))
    wqkvT = np.ascontiguousarray(np.asarray(w_qkv, np.float32).T)
    bv = np.ascontiguousarray(
        np.asarray(b_qkv, np.float32)[2 * IC:3 * IC].reshape(IC, 1)
    )
    woutT = np.ascontiguousarray(np.asarray(w_out, np.float32).T)
    bout = np.ascontiguousarray(np.asarray(b_out, np.float32).reshape(CH, 1))
    return [
        {
            "x": np.ascontiguousarray(xs[i]),
            "wqkvT": wqkvT,
            "bv": bv,
            "woutT": woutT,
            "bout": bout,
        }
        for i in range(8)
    ]


def kernel(x, w_qkv, b_qkv, w_out, b_out, _trace=False, _trace_kwargs=None):
    nc = _get_nc()
    in_maps = _prep_in_maps(x, w_qkv, b_qkv, w_out, b_out)
    res = run_bass_kernel_spmd(
        nc, in_maps, core_ids=list(range(8)), trace=_trace,
        **(_trace_kwargs or {}),
    )
    out = np.stack([res.results[i]["out"] for i in range(8)])
    out = out.reshape(8, CH, 64, 64).astype(np.float32)
    if _trace:
        return out, res
    return out


if __name__ == "__main__":
    rng = np.random.default_rng(0)
    x = rng.standard_normal((8, CH, 64, 64), dtype=np.float32)
    w_qkv = (rng.standard_normal((3 * IC, CH), dtype=np.float32) * 0.01)
    b_qkv = (rng.standard_normal((3 * IC,), dtype=np.float32) * 0.01)
    w_out = (rng.standard_normal((CH, IC), dtype=np.float32) * 0.01)
    b_out = (rng.standard_normal((CH,), dtype=np.float32) * 0.01)
    o = kernel(x, w_qkv=w_qkv, b_qkv=b_qkv, w_out=w_out, b_out=b_out)
    print(o.shape, o.dtype)
